# revision 3
# baseline (speedup 1.0000x reference)
"""Trainium2 Bass kernel for nn_ChannelPolyLayer.

out[b,o,x,y] = sum_c coeffs[b,o,c] * prod_v img[b,v,x,y] ** powers[c,v]
with degree<=3 trivariate monomials (20 coeffs), img channels (u,v,w).

Strategy
  - Data parallel over batch: 16 batches -> 8 cores x 2 batches.
  - Per core, the 2 batches are folded onto the partition axis:
    partitions 0..63 hold batch 0's pixel rows, 64..127 batch 1's.
    Per-partition coefficient APs then encode batch-dependent scalars,
    so a single SPMD program serves all cores and batches.
  - Exact factored evaluation (no pow):
        out_o = u*A_o(u,v,w) + v*D_o(v,w) + w*E_o(w) + c0_o
    A_o: inhomogeneous quadratic (10 coeffs), D_o: quadratic in (v,w) (6),
    E_o: quadratic in w (3), c0_o const.  Chains run as fused
    (mono*coeff)+acc scalar_tensor_tensor ops.
  - Work is split across engines: ScalarE does squares + chain heads,
    VectorE does the A-chains + products, GpSimd does crosses, D/E chains
    and the final sums, so the three engines run concurrently.
"""

import numpy as np

N_CORES = 8
BATCH, NVARS, H, W = 16, 3, 512, 512
NPIX = H * W            # 262144
P = 128
BPC = BATCH // N_CORES  # 2 batches per core
ROWS = P // BPC         # 64 partition rows per batch
COLS = NPIX // ROWS     # 4096 columns per plane
CW = 1024               # chunk width
NCHUNK = COLS // CW     # 4
NOUT = 3

# Coefficient-table layout per output o (20 columns each, 60 total):
# A (10): [const, u, v, w, u2, uv, uw, v2, vw, w2]
# D (6):  [const, v, w, v2, vw, w2]
# E (3):  [const, w, w2]
# c0 (1)
A_BASIS = [(0, 0, 0), (1, 0, 0), (0, 1, 0), (0, 0, 1), (2, 0, 0),
           (1, 1, 0), (1, 0, 1), (0, 2, 0), (0, 1, 1), (0, 0, 2)]
D_BASIS = [(0, 0, 0), (0, 1, 0), (0, 0, 1), (0, 2, 0), (0, 1, 1), (0, 0, 2)]
E_BASIS = [(0, 0, 0), (0, 0, 1), (0, 0, 2)]
TAB_COLS = NOUT * 20  # 60


def _coeff_table(coeffs_core: np.ndarray, powers: np.ndarray) -> np.ndarray:
    """coeffs_core [BPC, NOUT, 20] -> [P, TAB_COLS]; rows 0..63 batch0, 64.. batch1."""
    pw = [tuple(int(round(x)) for x in row) for row in np.asarray(powers)]
    tab = np.zeros((BPC, NOUT, 20), np.float64)
    a_idx = {m: i for i, m in enumerate(A_BASIS)}
    d_idx = {m: i for i, m in enumerate(D_BASIS)}
    e_idx = {m: i for i, m in enumerate(E_BASIS)}
    for b in range(BPC):
        for o in range(NOUT):
            for c, (pu, pv, pwz) in enumerate(pw):
                val = float(coeffs_core[b, o, c])
                if pu > 0:
                    tab[b, o, a_idx[(pu - 1, pv, pwz)]] += val
                elif pv > 0:
                    tab[b, o, 10 + d_idx[(0, pv - 1, pwz)]] += val
                elif pwz > 0:
                    tab[b, o, 16 + e_idx[(0, 0, pwz - 1)]] += val
                else:
                    tab[b, o, 19] += val
    out = np.empty((P, TAB_COLS), np.float32)
    for b in range(BPC):
        out[b * ROWS:(b + 1) * ROWS, :] = tab[b].reshape(1, TAB_COLS)
    return out


_NC_CACHE = {}

# Dev knobs (unused by the grading harness): extra kwargs forwarded to
# run_bass_kernel_spmd, and the last BassKernelResults for inspection.
RUN_KWARGS: dict = {}
LAST_RESULTS = None


def _build_nc():
    if "nc" in _NC_CACHE:
        return _NC_CACHE["nc"]
    import concourse.mybir as mybir
    from concourse import bacc
    from concourse.tile import TileContext

    F32 = mybir.dt.float32
    MUL = mybir.AluOpType.mult
    ADD = mybir.AluOpType.add
    IDENT = mybir.ActivationFunctionType.Identity

    nc = bacc.Bacc("TRN2", target_bir_lowering=False)
    img = nc.dram_tensor("img", [NVARS, P, COLS], F32, kind="ExternalInput")
    ctab = nc.dram_tensor("ctab", [P, TAB_COLS], F32, kind="ExternalInput")
    out = nc.dram_tensor("out", [NOUT, P, COLS], F32, kind="ExternalOutput")

    with TileContext(nc) as tc:
        with (
            tc.tile_pool(name="tabp", bufs=1) as tabp,
            tc.tile_pool(name="inp", bufs=2) as inp,
            tc.tile_pool(name="sqp", bufs=2) as sqp,
            tc.tile_pool(name="crp", bufs=2) as crp,
            tc.tile_pool(name="chain", bufs=1) as chain,
            tc.tile_pool(name="prod", bufs=1) as prod,
            tc.tile_pool(name="outp", bufs=2) as outp,
        ):
            tab = tabp.tile([P, TAB_COLS], F32)
            nc.sync.dma_start(out=tab, in_=ctab[:, :])

            def col(o, k):
                j = o * 20 + k
                return tab[:, j:j + 1]

            for chk in range(NCHUNK):
                c0, c1 = chk * CW, (chk + 1) * CW
                u = inp.tile([P, CW], F32, tag="u")
                v = inp.tile([P, CW], F32, tag="v")
                w = inp.tile([P, CW], F32, tag="w")
                nc.sync.dma_start(out=u, in_=img[0, :, c0:c1])
                nc.sync.dma_start(out=v, in_=img[1, :, c0:c1])
                nc.sync.dma_start(out=w, in_=img[2, :, c0:c1])

                u2 = sqp.tile([P, CW], F32, tag="u2")
                v2 = sqp.tile([P, CW], F32, tag="v2")
                w2 = sqp.tile([P, CW], F32, tag="w2")
                nc.scalar.square(u2, u)
                nc.scalar.square(v2, v)
                nc.scalar.square(w2, w)
                uv = crp.tile([P, CW], F32, tag="uv")
                uw = crp.tile([P, CW], F32, tag="uw")
                vw = crp.tile([P, CW], F32, tag="vw")
                nc.gpsimd.tensor_mul(out=uv, in0=u, in1=v)
                nc.gpsimd.tensor_mul(out=uw, in0=u, in1=w)
                nc.gpsimd.tensor_mul(out=vw, in0=v, in1=w)

                a_mono = [v, w, u2, uv, uw, v2, vw, w2]
                d_mono = [w, v2, vw, w2]

                ats, dts, ets = {}, {}, {}
                for o in range(NOUT):
                    # A/D/E chains: heads on ACT, bodies fused stt on DVE
                    at = chain.tile([P, CW], F32, tag=f"at{o}", name=f"at{o}")
                    nc.scalar.activation(out=at, in_=u, func=IDENT,
                                         bias=col(o, 0), scale=col(o, 1))
                    for i, m in enumerate(a_mono):
                        nc.vector.scalar_tensor_tensor(
                            out=at, in0=m, scalar=col(o, 2 + i),
                            in1=at, op0=MUL, op1=ADD)
                    dt_ = chain.tile([P, CW], F32, tag=f"dt{o}", name=f"dt{o}")
                    nc.scalar.activation(out=dt_, in_=v, func=IDENT,
                                         bias=col(o, 10), scale=col(o, 11))
                    for i, m in enumerate(d_mono):
                        nc.vector.scalar_tensor_tensor(
                            out=dt_, in0=m, scalar=col(o, 12 + i),
                            in1=dt_, op0=MUL, op1=ADD)
                    et = chain.tile([P, CW], F32, tag=f"et{o}", name=f"et{o}")
                    nc.scalar.activation(out=et, in_=w, func=IDENT,
                                         bias=col(o, 16), scale=col(o, 17))
                    nc.vector.scalar_tensor_tensor(
                        out=et, in0=w2, scalar=col(o, 18),
                        in1=et, op0=MUL, op1=ADD)
                    ats[o], dts[o], ets[o] = at, dt_, et

                for o in range(NOUT):
                    # products and partial sum on GpSimd, final fused add on DVE
                    p1 = prod.tile([P, CW], F32, tag=f"p1{o}", name=f"p1{o}")
                    p2 = prod.tile([P, CW], F32, tag=f"p2{o}", name=f"p2{o}")
                    p3 = prod.tile([P, CW], F32, tag=f"p3{o}", name=f"p3{o}")
                    nc.gpsimd.tensor_mul(out=p1, in0=u, in1=ats[o])
                    nc.gpsimd.tensor_mul(out=p2, in0=v, in1=dts[o])
                    nc.gpsimd.tensor_mul(out=p3, in0=w, in1=ets[o])
                    s = prod.tile([P, CW], F32, tag=f"s{o}", name=f"s{o}")
                    nc.gpsimd.tensor_add(out=s, in0=p1, in1=p2)
                    ot = outp.tile([P, CW], F32, tag=f"ot{o}", name=f"ot{o}")
                    nc.vector.scalar_tensor_tensor(
                        out=ot, in0=p3, scalar=col(o, 19),
                        in1=s, op0=ADD, op1=ADD)
                    nc.sync.dma_start(out=out[o, :, c0:c1], in_=ot)
    nc.finalize()
    _NC_CACHE["nc"] = nc
    return nc


def _shard_core(img: np.ndarray, c: int) -> np.ndarray:
    """img [BATCH,3,H,W] -> per-core [NVARS, P, COLS] with batch on partitions."""
    blk = np.empty((NVARS, P, COLS), np.float32)
    for b in range(BPC):
        plane = img[c * BPC + b].reshape(NVARS, ROWS, COLS)
        blk[:, b * ROWS:(b + 1) * ROWS, :] = plane
    return blk


def kernel(img: np.ndarray, coeffs: np.ndarray, powers: np.ndarray) -> np.ndarray:
    from concourse.bass_utils import run_bass_kernel_spmd

    img = np.ascontiguousarray(np.asarray(img, np.float32))
    coeffs = np.asarray(coeffs, np.float32)
    powers = np.asarray(powers, np.float32)

    nc = _build_nc()
    in_maps = []
    for c in range(N_CORES):
        in_maps.append({
            "img": _shard_core(img, c),
            "ctab": _coeff_table(coeffs[c * BPC:(c + 1) * BPC], powers),
        })

    res = run_bass_kernel_spmd(nc, in_maps, core_ids=list(range(N_CORES)),
                               **RUN_KWARGS)
    global LAST_RESULTS
    LAST_RESULTS = res
    out = np.empty((BATCH, NOUT, H, W), np.float32)
    for c in range(N_CORES):
        blk = res.results[c]["out"]  # [NOUT, P, COLS]
        for b in range(BPC):
            out[c * BPC + b] = blk[:, b * ROWS:(b + 1) * ROWS, :].reshape(NOUT, H, W)
    return out



# revision 5
# speedup vs baseline: 1.9581x; 1.9581x over previous
"""Trainium2 Bass kernel for nn_ChannelPolyLayer.

out[b,o,x,y] = sum_c coeffs[b,o,c] * prod_v img[b,v,x,y] ** powers[c,v]
with degree<=3 trivariate monomials (20 coeffs), img channels (u,v,w).

Strategy (v5)
  - Data parallel over batch: 16 batches -> 8 cores x 2 batches; per core
    the 2 batches are folded onto the partition axis (rows 0..63 batch0,
    64..127 batch1), so per-partition scalar APs encode batch-dependent
    coefficients and one SPMD program serves all cores.
  - Factored evaluation  out = T(w) + u*A(u,v,w) + v*D(v,w)  where
        A: inhomogeneous quadratic (10 coeffs), D: quadratic in (v,w) (6),
        T: cubic in w alone incl. the global constant (4).
  - Completing-the-square: each (x^2, x) coefficient pair of A/D/T is
    evaluated as lam*Square(x + beta) + delta on the Scalar engine
    (f32 output; the cancellation lam*beta^2 is folded into delta inside a
    single fused tensor_scalar, so bf16 only ever rounds the small result).
  - Everything else runs as bf16 tensor_scalar leaves (DVE 4x mode) and
    bf16 tensor_tensor adds/mults (DVE 2x mode). No scalar_tensor_tensor
    (1x only on DVE) and no GpSimd (SBUF port contention poisons DVE).
  - ACT and DVE streams are balanced: ACT does the 18 squares + w^2 and
    most folds (Identity), DVE does cross/monomial leaves, trees, products.
"""

import numpy as np
import ml_dtypes

N_CORES = 8
BATCH, NVARS, H, W = 16, 3, 512, 512
NPIX = H * W            # 262144
P = 128
BPC = BATCH // N_CORES  # 2 batches per core
ROWS = P // BPC         # 64 partition rows per batch
COLS = NPIX // ROWS     # 4096 columns per plane
CW = 2048               # chunk width
NCHUNK = COLS // CW     # 2
NOUT = 3

# ctab column layout per output o (23 columns each, 69 total):
#   0..5   beta  for squares  [Au, Av, Aw, Dv, Dw, Tw]
#   6..11  lam   for folds    [same order]
#   12..17 delta for folds    [same order]
#   18 c_uv(A) 19 c_uw(A) 20 c_vw(A) 21 c_vw(D) 22 c_w3(T)
TAB_PER_OUT = 23
TAB_COLS = NOUT * TAB_PER_OUT

# folds whose Identity runs on ACT instead of a DVE tensor_scalar
# (balance valve: ACT ~2.09us/op, DVE f32-in ts ~1.35us/op per 2048-chunk)
ACT_FOLDS = {(0, 0), (0, 1), (0, 2), (0, 3), (0, 4), (0, 5),
             (1, 0), (1, 1), (1, 2), (1, 3), (1, 4), (1, 5),
             (2, 5)}


def _fold_params(quad, lin, delta):
    """lam*Square(x+beta) + dlt  ==  quad*x^2 + lin*x + delta  (f64 host)."""
    aq, al = abs(quad), abs(lin)
    if aq < 1e-12 and al < 1e-12:
        return 0.0, 0.0, delta
    q = quad if aq >= al / 2000.0 else (al / 2000.0 if quad >= 0 else -al / 2000.0)
    beta = lin / (2.0 * q)
    lam = q
    return beta, lam, delta - lam * beta * beta


def _coeff_table(coeffs_core: np.ndarray, powers: np.ndarray) -> np.ndarray:
    """coeffs_core [BPC, NOUT, 20] -> ctab [P, TAB_COLS] f32."""
    pw = [tuple(int(round(x)) for x in row) for row in np.asarray(powers)]
    amap = {(0, 0, 0): 0, (1, 0, 0): 1, (0, 1, 0): 2, (0, 0, 1): 3,
            (2, 0, 0): 4, (1, 1, 0): 5, (1, 0, 1): 6, (0, 2, 0): 7,
            (0, 1, 1): 8, (0, 0, 2): 9}
    dmap = {(0, 0, 0): 0, (0, 1, 0): 1, (0, 0, 1): 2, (0, 2, 0): 3,
            (0, 1, 1): 4, (0, 0, 2): 5}
    out = np.empty((P, TAB_COLS), np.float32)
    for b in range(BPC):
        row = np.zeros(TAB_COLS, np.float64)
        for o in range(NOUT):
            A = np.zeros(10); D = np.zeros(6); T = np.zeros(4)
            for c, (pu, pv, pz) in enumerate(pw):
                val = float(coeffs_core[b, o, c])
                if pu > 0:
                    A[amap[(pu - 1, pv, pz)]] += val
                elif pv > 0:
                    D[dmap[(0, pv - 1, pz)]] += val
                else:
                    T[pz] += val
            base = o * TAB_PER_OUT
            pairs = [(A[4], A[1], A[0]),   # Au: u^2,u, const c100
                     (A[7], A[2], 0.0),    # Av
                     (A[9], A[3], 0.0),    # Aw
                     (D[3], D[1], D[0]),   # Dv: const c010
                     (D[5], D[2], 0.0),    # Dw
                     (T[2], T[1], T[0])]   # Tw: const c000
            for i, (q, l, d) in enumerate(pairs):
                beta, lam, dlt = _fold_params(q, l, d)
                row[base + i] = beta
                row[base + 6 + i] = lam
                row[base + 12 + i] = dlt
            row[base + 18] = A[5]  # uv
            row[base + 19] = A[6]  # uw
            row[base + 20] = A[8]  # vw (A)
            row[base + 21] = D[4]  # vw (D)
            row[base + 22] = T[3]  # w3
        out[b * ROWS:(b + 1) * ROWS, :] = row.astype(np.float32)
    return out


_NC_CACHE = {}

# Dev knobs (unused by the grading harness): extra kwargs forwarded to
# run_bass_kernel_spmd, and the last BassKernelResults for inspection.
RUN_KWARGS: dict = {}
LAST_RESULTS = None


def _build_nc():
    if "nc" in _NC_CACHE:
        return _NC_CACHE["nc"]
    import concourse.mybir as mybir
    from concourse import bacc
    from concourse.tile import TileContext

    F32 = mybir.dt.float32
    BF16 = mybir.dt.bfloat16
    MUL = mybir.AluOpType.mult
    ADD = mybir.AluOpType.add
    IDENT = mybir.ActivationFunctionType.Identity
    SQ = mybir.ActivationFunctionType.Square

    nc = bacc.Bacc("TRN2", target_bir_lowering=False)
    img = nc.dram_tensor("img", [NVARS, P, COLS], BF16, kind="ExternalInput")
    ctab = nc.dram_tensor("ctab", [P, TAB_COLS], F32, kind="ExternalInput")
    out = nc.dram_tensor("out", [NOUT, P, COLS], BF16, kind="ExternalOutput")

    with TileContext(nc) as tc:
        with (
            tc.tile_pool(name="tabp", bufs=1) as tabp,
            tc.tile_pool(name="inp", bufs=2) as inp,
            tc.tile_pool(name="crs", bufs=2) as crs,
            tc.tile_pool(name="sqp", bufs=4) as sqp,
            tc.tile_pool(name="leafp", bufs=12) as leafp,
            tc.tile_pool(name="accp", bufs=6) as accp,
            tc.tile_pool(name="outp", bufs=2) as outp,
        ):
            tab = tabp.tile([P, TAB_COLS], F32)
            nc.sync.dma_start(out=tab, in_=ctab[:, :])

            def col(o, k):
                j = o * TAB_PER_OUT + k
                return tab[:, j:j + 1]

            for chk in range(NCHUNK):
                c0, c1 = chk * CW, (chk + 1) * CW
                ub = inp.tile([P, CW], BF16, tag="ub")
                vb = inp.tile([P, CW], BF16, tag="vb")
                wb = inp.tile([P, CW], BF16, tag="wb")
                nc.sync.dma_start(out=ub, in_=img[0, :, c0:c1])
                nc.sync.dma_start(out=vb, in_=img[1, :, c0:c1])
                nc.sync.dma_start(out=wb, in_=img[2, :, c0:c1])

                w2b = crs.tile([P, CW], BF16, tag="w2b")
                nc.scalar.activation(out=w2b, in_=wb, func=SQ)
                uv = crs.tile([P, CW], BF16, tag="uv")
                uw = crs.tile([P, CW], BF16, tag="uw")
                vw = crs.tile([P, CW], BF16, tag="vw")
                w3 = crs.tile([P, CW], BF16, tag="w3")
                nc.vector.tensor_tensor(out=uv, in0=ub, in1=vb, op=MUL)
                nc.vector.tensor_tensor(out=uw, in0=ub, in1=wb, op=MUL)
                nc.vector.tensor_tensor(out=vw, in0=vb, in1=wb, op=MUL)
                nc.vector.tensor_tensor(out=w3, in0=w2b, in1=wb, op=MUL)

                sq_in = [ub, vb, wb, vb, wb, wb]  # Au Av Aw Dv Dw Tw
                for o in range(NOUT):
                    # squares on ACT (f32 out), folds fused lam*sq+delta
                    folds = []
                    for i in range(6):
                        sq = sqp.tile([P, CW], F32, tag="sq",
                                      name=f"sq{o}_{i}_{chk}")
                        nc.scalar.activation(out=sq, in_=sq_in[i], func=SQ,
                                             bias=col(o, i))
                        fl = leafp.tile([P, CW], BF16, tag="leaf",
                                        name=f"fold{o}_{i}_{chk}")
                        if (o, i) in ACT_FOLDS:
                            nc.scalar.activation(out=fl, in_=sq, func=IDENT,
                                                 scale=col(o, 6 + i),
                                                 bias=col(o, 12 + i))
                        else:
                            nc.vector.tensor_scalar(
                                out=fl, in0=sq, scalar1=col(o, 6 + i),
                                scalar2=col(o, 12 + i), op0=MUL, op1=ADD)
                        folds.append(fl)

                    def leaf(src, k, nm):
                        t = leafp.tile([P, CW], BF16, tag="leaf", name=nm)
                        nc.vector.tensor_scalar(out=t, in0=src,
                                                scalar1=col(o, k),
                                                scalar2=0.0, op0=MUL, op1=ADD)
                        return t

                    luv = leaf(uv, 18, f"luv{o}_{chk}")
                    luw = leaf(uw, 19, f"luw{o}_{chk}")
                    lvwA = leaf(vw, 20, f"lvwA{o}_{chk}")
                    lvwD = leaf(vw, 21, f"lvwD{o}_{chk}")
                    lw3 = leaf(w3, 22, f"lw3{o}_{chk}")

                    def tt(a, b, nm, op=ADD, pool=accp, tag="acc"):
                        t = pool.tile([P, CW], BF16, tag=tag, name=nm)
                        nc.vector.tensor_tensor(out=t, in0=a, in1=b, op=op)
                        return t

                    a1 = tt(folds[0], folds[1], f"a1_{o}_{chk}")
                    a2 = tt(folds[2], luv, f"a2_{o}_{chk}")
                    a3 = tt(luw, lvwA, f"a3_{o}_{chk}")
                    a4 = tt(a1, a2, f"a4_{o}_{chk}")
                    at = tt(a4, a3, f"at_{o}_{chk}")
                    d1 = tt(folds[3], folds[4], f"d1_{o}_{chk}")
                    dt = tt(d1, lvwD, f"dt_{o}_{chk}")
                    tt_ = tt(folds[5], lw3, f"tt_{o}_{chk}")
                    p1 = tt(at, ub, f"p1_{o}_{chk}", op=MUL)
                    p2 = tt(dt, vb, f"p2_{o}_{chk}", op=MUL)
                    s = tt(p1, p2, f"s_{o}_{chk}")
                    ot = outp.tile([P, CW], BF16, tag=f"ot{o}",
                                   name=f"ot_{o}_{chk}")
                    nc.vector.tensor_tensor(out=ot, in0=s, in1=tt_, op=ADD)
                    nc.sync.dma_start(out=out[o, :, c0:c1], in_=ot)
    nc.finalize()
    _NC_CACHE["nc"] = nc
    return nc


def _shard_core(img_bf: np.ndarray, c: int) -> np.ndarray:
    """img_bf [BATCH,3,H,W] bf16 -> per-core [NVARS, P, COLS]."""
    blk = np.empty((NVARS, P, COLS), ml_dtypes.bfloat16)
    for b in range(BPC):
        plane = img_bf[c * BPC + b].reshape(NVARS, ROWS, COLS)
        blk[:, b * ROWS:(b + 1) * ROWS, :] = plane
    return blk


def kernel(img: np.ndarray, coeffs: np.ndarray, powers: np.ndarray) -> np.ndarray:
    from concourse.bass_utils import run_bass_kernel_spmd

    img_bf = np.asarray(img, np.float32).astype(ml_dtypes.bfloat16)
    coeffs = np.asarray(coeffs, np.float32)
    powers = np.asarray(powers, np.float32)

    nc = _build_nc()
    in_maps = []
    for c in range(N_CORES):
        in_maps.append({
            "img": _shard_core(img_bf, c),
            "ctab": _coeff_table(coeffs[c * BPC:(c + 1) * BPC], powers),
        })

    res = run_bass_kernel_spmd(nc, in_maps, core_ids=list(range(N_CORES)),
                               **RUN_KWARGS)
    global LAST_RESULTS
    LAST_RESULTS = res
    out = np.empty((BATCH, NOUT, H, W), np.float32)
    for c in range(N_CORES):
        blk = np.asarray(res.results[c]["out"], dtype=np.float32)
        for b in range(BPC):
            out[c * BPC + b] = blk[:, b * ROWS:(b + 1) * ROWS, :].reshape(
                NOUT, H, W)
    return out


# revision 8
# speedup vs baseline: 2.3287x; 1.1893x over previous
"""Trainium2 Bass kernel for nn_ChannelPolyLayer.

out[b,o,x,y] = sum_c coeffs[b,o,c] * prod_v img[b,v,x,y] ** powers[c,v]
with degree<=3 trivariate monomials (20 coeffs), img channels (u,v,w).

Strategy (v5)
  - Data parallel over batch: 16 batches -> 8 cores x 2 batches; per core
    the 2 batches are folded onto the partition axis (rows 0..63 batch0,
    64..127 batch1), so per-partition scalar APs encode batch-dependent
    coefficients and one SPMD program serves all cores.
  - Factored evaluation  out = T(w) + u*A(u,v,w) + v*D(v,w)  where
        A: inhomogeneous quadratic (10 coeffs), D: quadratic in (v,w) (6),
        T: cubic in w alone incl. the global constant (4).
  - Completing-the-square: each (x^2, x) coefficient pair of A/D/T is
    evaluated as lam*Square(x + beta) + delta on the Scalar engine
    (f32 output; the cancellation lam*beta^2 is folded into delta inside a
    single fused tensor_scalar, so bf16 only ever rounds the small result).
  - Everything else runs as bf16 tensor_scalar leaves (DVE 4x mode) and
    bf16 tensor_tensor adds/mults (DVE 2x mode). No scalar_tensor_tensor
    (1x only on DVE) and no GpSimd (SBUF port contention poisons DVE).
  - ACT and DVE streams are balanced: ACT does the 18 squares + w^2 and
    most folds (Identity), DVE does cross/monomial leaves, trees, products.
"""

import numpy as np
import ml_dtypes

N_CORES = 8
BATCH, NVARS, H, W = 16, 3, 512, 512
NPIX = H * W            # 262144
P = 128
BPC = BATCH // N_CORES  # 2 batches per core
ROWS = P // BPC         # 64 partition rows per batch
COLS = NPIX // ROWS     # 4096 columns per plane
CW = 2048               # chunk width
NCHUNK = COLS // CW     # 2
NOUT = 3

# ctab column layout per output o (23 columns each, 69 total):
#   0..5   beta  for squares  [Au, Av, Aw, Dv, Dw, Tw]
#   6..11  lam   for folds    [same order]
#   12..17 delta for folds    [same order]
#   18 c_uv(A) 19 c_uw(A) 20 c_vw(A) 21 c_vw(D) 22 c_w3(T)
TAB_PER_OUT = 23
TAB_COLS = NOUT * TAB_PER_OUT

# folds whose Identity runs on ACT instead of a DVE tensor_scalar
# (balance valve: ACT ~2.09us/op, DVE f32-in ts ~1.35us/op per 2048-chunk)
ACT_FOLDS = {(1, 0), (1, 1), (1, 2), (1, 3), (1, 4), (1, 5),
             (2, 0), (2, 1), (2, 2), (2, 3), (2, 4), (2, 5),
             (0, 5)}


def _fold_params(quad, lin, delta):
    """lam*Square(x+beta) + dlt  ==  quad*x^2 + lin*x + delta  (f64 host)."""
    aq, al = abs(quad), abs(lin)
    if aq < 1e-12 and al < 1e-12:
        return 0.0, 0.0, delta
    q = quad if aq >= al / 2000.0 else (al / 2000.0 if quad >= 0 else -al / 2000.0)
    beta = lin / (2.0 * q)
    lam = q
    return beta, lam, delta - lam * beta * beta


def _coeff_table(coeffs_core: np.ndarray, powers: np.ndarray) -> np.ndarray:
    """coeffs_core [BPC, NOUT, 20] -> ctab [P, TAB_COLS] f32."""
    pw = [tuple(int(round(x)) for x in row) for row in np.asarray(powers)]
    amap = {(0, 0, 0): 0, (1, 0, 0): 1, (0, 1, 0): 2, (0, 0, 1): 3,
            (2, 0, 0): 4, (1, 1, 0): 5, (1, 0, 1): 6, (0, 2, 0): 7,
            (0, 1, 1): 8, (0, 0, 2): 9}
    dmap = {(0, 0, 0): 0, (0, 1, 0): 1, (0, 0, 1): 2, (0, 2, 0): 3,
            (0, 1, 1): 4, (0, 0, 2): 5}
    out = np.empty((P, TAB_COLS), np.float32)
    for b in range(BPC):
        row = np.zeros(TAB_COLS, np.float64)
        for o in range(NOUT):
            A = np.zeros(10); D = np.zeros(6); T = np.zeros(4)
            for c, (pu, pv, pz) in enumerate(pw):
                val = float(coeffs_core[b, o, c])
                if pu > 0:
                    A[amap[(pu - 1, pv, pz)]] += val
                elif pv > 0:
                    D[dmap[(0, pv - 1, pz)]] += val
                else:
                    T[pz] += val
            base = o * TAB_PER_OUT
            pairs = [(A[4], A[1], A[0]),   # Au: u^2,u, const c100
                     (A[7], A[2], 0.0),    # Av
                     (A[9], A[3], 0.0),    # Aw
                     (D[3], D[1], D[0]),   # Dv: const c010
                     (D[5], D[2], 0.0),    # Dw
                     (T[2], T[1], T[0])]   # Tw: const c000
            for i, (q, l, d) in enumerate(pairs):
                beta, lam, dlt = _fold_params(q, l, d)
                row[base + i] = beta
                row[base + 6 + i] = lam
                row[base + 12 + i] = dlt
            row[base + 18] = A[5]  # uv
            row[base + 19] = A[6]  # uw
            row[base + 20] = A[8]  # vw (A)
            row[base + 21] = D[4]  # vw (D)
            row[base + 22] = T[3]  # w3
        out[b * ROWS:(b + 1) * ROWS, :] = row.astype(np.float32)
    return out


_NC_CACHE = {}

# Dev knobs (unused by the grading harness): extra kwargs forwarded to
# run_bass_kernel_spmd, and the last BassKernelResults for inspection.
RUN_KWARGS: dict = {}
LAST_RESULTS = None


def _build_nc():
    if "nc" in _NC_CACHE:
        return _NC_CACHE["nc"]
    import concourse.mybir as mybir
    from concourse import bacc
    from concourse.tile import TileContext

    F32 = mybir.dt.float32
    BF16 = mybir.dt.bfloat16
    MUL = mybir.AluOpType.mult
    ADD = mybir.AluOpType.add
    IDENT = mybir.ActivationFunctionType.Identity
    SQ = mybir.ActivationFunctionType.Square

    nc = bacc.Bacc("TRN2", target_bir_lowering=False)
    img = nc.dram_tensor("img", [NVARS, P, COLS], BF16, kind="ExternalInput")
    ctab = nc.dram_tensor("ctab", [P, TAB_COLS], F32, kind="ExternalInput")
    out = nc.dram_tensor("out", [NOUT, P, COLS], BF16, kind="ExternalOutput")

    with TileContext(nc) as tc:
        with (
            tc.tile_pool(name="tabp", bufs=1) as tabp,
            tc.tile_pool(name="inp", bufs=2) as inp,
            tc.tile_pool(name="crs", bufs=2) as crs,
            tc.tile_pool(name="sqp", bufs=4) as sqp,
            tc.tile_pool(name="leafp", bufs=6) as leafp,
            tc.tile_pool(name="accp", bufs=6) as accp,
            tc.tile_pool(name="outp", bufs=2) as outp,
        ):
            tab = tabp.tile([P, TAB_COLS], F32)
            nc.sync.dma_start(out=tab, in_=ctab[:, :])

            def col(o, k):
                j = o * TAB_PER_OUT + k
                return tab[:, j:j + 1]

            for chk in range(NCHUNK):
                c0, c1 = chk * CW, (chk + 1) * CW
                ub = inp.tile([P, CW], BF16, tag="ub")
                vb = inp.tile([P, CW], BF16, tag="vb")
                wb = inp.tile([P, CW], BF16, tag="wb")
                nc.sync.dma_start(out=ub, in_=img[0, :, c0:c1])
                nc.sync.dma_start(out=vb, in_=img[1, :, c0:c1])
                nc.sync.dma_start(out=wb, in_=img[2, :, c0:c1])

                w2b = crs.tile([P, CW], BF16, tag="w2b")
                nc.scalar.activation(out=w2b, in_=wb, func=SQ)
                uv = crs.tile([P, CW], BF16, tag="uv")
                uw = crs.tile([P, CW], BF16, tag="uw")
                vw = crs.tile([P, CW], BF16, tag="vw")
                w3 = crs.tile([P, CW], BF16, tag="w3")
                nc.vector.tensor_tensor(out=uv, in0=ub, in1=vb, op=MUL)
                nc.vector.tensor_tensor(out=uw, in0=ub, in1=wb, op=MUL)
                nc.vector.tensor_tensor(out=vw, in0=vb, in1=wb, op=MUL)
                nc.vector.tensor_tensor(out=w3, in0=w2b, in1=wb, op=MUL)

                sq_in = [ub, vb, wb, vb, wb, wb]  # Au Av Aw Dv Dw Tw
                for o in range(NOUT):
                    # squares on ACT (f32 out), folds fused lam*sq+delta
                    folds = []
                    for i in range(6):
                        sq = sqp.tile([P, CW], F32, tag="sq",
                                      name=f"sq{o}_{i}_{chk}")
                        nc.scalar.activation(out=sq, in_=sq_in[i], func=SQ,
                                             bias=col(o, i))
                        fl = leafp.tile([P, CW], BF16, tag="leaf",
                                        name=f"fold{o}_{i}_{chk}")
                        if (o, i) in ACT_FOLDS:
                            nc.scalar.activation(out=fl, in_=sq, func=IDENT,
                                                 scale=col(o, 6 + i),
                                                 bias=col(o, 12 + i))
                        else:
                            nc.vector.tensor_scalar(
                                out=fl, in0=sq, scalar1=col(o, 6 + i),
                                scalar2=col(o, 12 + i), op0=MUL, op1=ADD)
                        folds.append(fl)

                    def leaf(src, k, nm):
                        t = leafp.tile([P, CW], BF16, tag="leaf", name=nm)
                        nc.vector.tensor_scalar(out=t, in0=src,
                                                scalar1=col(o, k),
                                                scalar2=0.0, op0=MUL, op1=ADD)
                        return t

                    def tt(a, b, nm, op=ADD, pool=accp, tag="acc"):
                        t = pool.tile([P, CW], BF16, tag=tag, name=nm)
                        nc.vector.tensor_tensor(out=t, in0=a, in1=b, op=op)
                        return t

                    # running chains; leaves emitted adjacent to their use
                    at = tt(folds[0], folds[1], f"a1_{o}_{chk}")
                    at = tt(at, folds[2], f"a2_{o}_{chk}")
                    luv = leaf(uv, 18, f"luv{o}_{chk}")
                    at = tt(at, luv, f"a3_{o}_{chk}")
                    luw = leaf(uw, 19, f"luw{o}_{chk}")
                    at = tt(at, luw, f"a4_{o}_{chk}")
                    lvwA = leaf(vw, 20, f"lvwA{o}_{chk}")
                    at = tt(at, lvwA, f"a5_{o}_{chk}")
                    dt = tt(folds[3], folds[4], f"d1_{o}_{chk}")
                    lvwD = leaf(vw, 21, f"lvwD{o}_{chk}")
                    dt = tt(dt, lvwD, f"d2_{o}_{chk}")
                    lw3 = leaf(w3, 22, f"lw3{o}_{chk}")
                    tt_ = tt(folds[5], lw3, f"tt_{o}_{chk}")
                    p1 = tt(at, ub, f"p1_{o}_{chk}", op=MUL)
                    p2 = tt(dt, vb, f"p2_{o}_{chk}", op=MUL)
                    s = tt(p1, p2, f"s_{o}_{chk}")
                    ot = outp.tile([P, CW], BF16, tag=f"ot{o}",
                                   name=f"ot_{o}_{chk}")
                    nc.vector.tensor_tensor(out=ot, in0=s, in1=tt_, op=ADD)
                    nc.sync.dma_start(out=out[o, :, c0:c1], in_=ot)
    nc.finalize()
    _NC_CACHE["nc"] = nc
    return nc


def _shard_core(img_bf: np.ndarray, c: int) -> np.ndarray:
    """img_bf [BATCH,3,H,W] bf16 -> per-core [NVARS, P, COLS]."""
    blk = np.empty((NVARS, P, COLS), ml_dtypes.bfloat16)
    for b in range(BPC):
        plane = img_bf[c * BPC + b].reshape(NVARS, ROWS, COLS)
        blk[:, b * ROWS:(b + 1) * ROWS, :] = plane
    return blk


def kernel(img: np.ndarray, coeffs: np.ndarray, powers: np.ndarray) -> np.ndarray:
    from concourse.bass_utils import run_bass_kernel_spmd

    img_bf = np.asarray(img, np.float32).astype(ml_dtypes.bfloat16)
    coeffs = np.asarray(coeffs, np.float32)
    powers = np.asarray(powers, np.float32)

    nc = _build_nc()
    in_maps = []
    for c in range(N_CORES):
        in_maps.append({
            "img": _shard_core(img_bf, c),
            "ctab": _coeff_table(coeffs[c * BPC:(c + 1) * BPC], powers),
        })

    res = run_bass_kernel_spmd(nc, in_maps, core_ids=list(range(N_CORES)),
                               **RUN_KWARGS)
    global LAST_RESULTS
    LAST_RESULTS = res
    out = np.empty((BATCH, NOUT, H, W), np.float32)
    for c in range(N_CORES):
        blk = np.asarray(res.results[c]["out"], dtype=np.float32)
        for b in range(BPC):
            out[c * BPC + b] = blk[:, b * ROWS:(b + 1) * ROWS, :].reshape(
                NOUT, H, W)
    return out
